# revision 12
# baseline (speedup 1.0000x reference)
"""Trainium2 Bass kernel for nn_Attention (dense transformer attention block).

Strategy: tensor-parallel across heads (2 heads per core on 8 cores).
Each core computes QKV -> QK-RMSNorm -> RoPE -> SDPA for its heads over the
full sequence, then an AllToAll redistributes pre-projection activations so
each core computes the full output projection (+bias) for its own slice of
rows.  Host-side assembly is a pure concat.

Layouts (per core, HL=2 local heads, D=64 head dim):
  - q/k are produced TRANSPOSED: [feature(128 partitions), token] where the
    partition order is [h0_even(32) h0_odd(32) | h1_even(32) h1_odd(32)]
    (even/odd = rotary real/imag feature pairs; realized by permuting the
    qkv_w rows on the host).  This makes the QK matmul a single contiguous
    K=64 matmul per head (legal base partitions 0/64) and RoPE a handful of
    partition-block elementwise ops against 4x-stacked cos/sin tables.
  - v is produced naturally: [token(128 partitions), feature] with a ones
    column appended per head so the AV matmul also produces softmax
    denominators.
  - softmax: non-causal, scores bounded (|s|<~8 after RMS norm), so
    exp without max-subtraction; normalization applied after AV.
All matmul operands bf16 (fp32 PSUM accumulation); statistics in fp32.
"""

import numpy as np

import concourse.bass as bass
import concourse.mybir as mybir
import concourse.tile as tile
from concourse import bacc
from concourse import bass_utils

F32 = mybir.dt.float32
BF16 = mybir.dt.bfloat16
AF = mybir.ActivationFunctionType

D = 64        # head dim
DH = D // 2   # rotary pairs per head
EPS = 1e-6

CFG_FULL = dict(B=2, N=2048, C=1024, R=8)


# --------------------------------------------------------------------------
# program builder
# --------------------------------------------------------------------------

def build_program(B, N, C, R, qk_row_pack=False):
    """Build the per-core Bass program (SPMD: same program on all R cores)."""
    H = C // D            # total heads
    HL = H // R           # heads per core
    assert HL == 2, "layout assumes 2 heads per core"
    CC = C // 128         # contraction chunks
    NT = B * N            # total tokens
    RT = NT // R          # rows per rank  == SDPA nq tile
    NQ = RT
    QT = min(512, N)      # qkv token-tile
    TPB = N // QT         # qkv tiles per batch
    NKC = N // 128        # nk chunks per batch
    QPB = N // NQ         # nq tiles per batch
    RPB = R // B          # ranks per batch
    VW = HL * 65          # v1 bytes-per-chunk block width ([64 data | 1] per head)
    GRP = 3               # score chunks per exp call (psum: 2*3 + av 2 = 8 banks)

    OT = min(512, C)      # proj output tile
    assert N % QT == 0 and N % 128 == 0 and N % NQ == 0
    assert C % OT == 0 and RT % 128 == 0 and R % B == 0
    assert R * 128 == C, "proj f-chunks assume R*128 == C"

    nc = bacc.Bacc("TRN2", target_bir_lowering=False, debug=False,
                   enable_asserts=False, num_devices=R)

    # I/O ------------------------------------------------------------------
    xT = nc.dram_tensor("xT", [C, NT], F32, kind="ExternalInput").ap()
    wqkT = nc.dram_tensor("wqkT", [C, 2 * HL * D], F32, kind="ExternalInput").ap()
    wvT = nc.dram_tensor("wvT", [C, HL * D], F32, kind="ExternalInput").ap()
    wpT = nc.dram_tensor("wpT", [C, C], F32, kind="ExternalInput").ap()
    biasT = nc.dram_tensor("biasT", [1, C], F32, kind="ExternalInput").ap()
    pos4T = nc.dram_tensor("pos4T", [128, N], F32, kind="ExternalInput").ap()
    wpq = nc.dram_tensor("wpq", [128, 1], F32, kind="ExternalInput").ap()
    wpk = nc.dram_tensor("wpk", [128, 1], F32, kind="ExternalInput").ap()
    selin = nc.dram_tensor("selin", [HL, 128], F32, kind="ExternalInput").ap()
    y = nc.dram_tensor("y", [RT, C], F32, kind="ExternalOutput").ap()

    # internal DRAM for the collective
    a2a_in = nc.dram_tensor("a2a_in", [R * 128, RT], BF16)
    a2a_out = nc.dram_tensor("a2a_out", [R * 128, RT], BF16)

    with tile.TileContext(nc) as tc:
        import contextlib
        octx = contextlib.ExitStack()
        with octx:
            const = octx.enter_context(tc.tile_pool(name="const", bufs=1))
            setup_ctx = contextlib.ExitStack()
            stage = setup_ctx.enter_context(tc.tile_pool(name="stage", bufs=2))

            # ---- constants / weights into SBUF --------------------------
            wqk_f = stage.tile([128, CC * 2 * HL * D], F32, tag="wstage")
            nc.sync.dma_start(
                out=wqk_f.rearrange("p (cc f) -> p cc f", cc=CC),
                in_=wqkT.rearrange("(cc p) f -> p cc f", p=128))
            wqk_bf = const.tile([128, CC * 2 * HL * D], BF16)
            nc.vector.tensor_copy(wqk_bf[:, :], wqk_f[:, :])
            wqk_v = wqk_bf.rearrange("p (cc f) -> p cc f", cc=CC)

            wv_f = stage.tile([128, CC * HL * D], F32, tag="wstage")
            nc.sync.dma_start(
                out=wv_f.rearrange("p (cc f) -> p cc f", cc=CC),
                in_=wvT.rearrange("(cc p) f -> p cc f", p=128))
            wv_bf = const.tile([128, CC * HL * D], BF16)
            nc.vector.tensor_copy(wv_bf[:, :], wv_f[:, :])
            wv_v = wv_bf.rearrange("p (cc f) -> p cc f", cc=CC)

            wp_f = stage.tile([128, CC * C], F32, tag="wpstage", bufs=1)
            nc.sync.dma_start(
                out=wp_f.rearrange("p (cc f) -> p cc f", cc=CC),
                in_=wpT.rearrange("(cc p) f -> p cc f", p=128))
            wp_bf = const.tile([128, CC * C], BF16)
            nc.vector.tensor_copy(wp_bf[:, :], wp_f[:, :])
            wp_v = wp_bf.rearrange("p (cc f) -> p cc f", cc=CC)

            bias_f = const.tile([1, C], F32)
            nc.sync.dma_start(out=bias_f[:, :], in_=biasT[:, :])
            bias_bf = const.tile([1, C], BF16)
            nc.vector.tensor_copy(bias_bf[:, :], bias_f[:, :])

            # rotary tables: cos = sin(x + pi/2)
            zero1 = const.tile([128, 1], F32)
            nc.vector.memset(zero1[:, :], 0.0)
            halfpi = const.tile([128, 1], F32)
            nc.vector.memset(halfpi[:, :], float(np.pi / 2))
            epsb = const.tile([128, 1], F32)
            nc.vector.memset(epsb[:, :], EPS)

            p4 = stage.tile([128, N], F32, tag="p4", bufs=1)
            nc.sync.dma_start(out=p4[:, :], in_=pos4T[:, :])
            sin4 = const.tile([128, N], F32)
            cos4 = const.tile([128, N], F32)
            nc.scalar.activation(sin4[:, :], p4[:, :], AF.Sin,
                                 bias=zero1[:, :])
            nc.scalar.activation(cos4[:, :], p4[:, :], AF.Sin,
                                 bias=halfpi[:, :])

            # norm weights (per-partition, permuted on host)
            wq_sb = const.tile([128, 1], F32)
            wk_sb = const.tile([128, 1], F32)
            nc.sync.dma_start(out=wq_sb[:, :], in_=wpq[:, :])
            nc.sync.dma_start(out=wk_sb[:, :], in_=wpk[:, :])

            # 1/w^2 block-diagonal selectors for the sum-of-squares matmul
            def make_invw2blk(w_sb, name):
                iw = stage.tile([128, 1], F32, tag="iw", name=f"iw_{name}")
                nc.vector.tensor_mul(iw[:, :], w_sb[:, :], w_sb[:, :])
                nc.vector.reciprocal(iw[:, :], iw[:, :])
                blk = const.tile([128, HL], F32, name=f"blk_{name}")
                nc.vector.memset(blk[:, :], 0.0)
                for h in range(HL):
                    nc.vector.tensor_copy(
                        blk[h * 64:(h + 1) * 64, h:h + 1],
                        iw[h * 64:(h + 1) * 64, :])
                blk_bf = const.tile([128, HL], BF16, name=f"blkbf_{name}")
                nc.vector.tensor_copy(blk_bf[:, :], blk[:, :])
                return blk_bf

            blkq_bf = make_invw2blk(wq_sb, "q")
            blkk_bf = make_invw2blk(wk_sb, "k")

            # head-selector for broadcasting rstd [HL, n] -> [128, n]
            sel_f = stage.tile([HL, 128], F32, tag="iw", name="sel_f")
            nc.sync.dma_start(out=sel_f[:, :], in_=selin[:, :])
            sel_bf = const.tile([HL, 128], BF16)
            nc.vector.tensor_copy(sel_bf[:, :], sel_f[:, :])
            setup_ctx.close()

            ones1 = const.tile([1, 128], BF16)
            nc.vector.memset(ones1[:, :], 1.0)

            # rope sign vector: -1 on r-blocks, +1 on i-blocks
            signv = const.tile([128, 1], F32)
            nc.vector.memset(signv[:, :], 1.0)
            for h in range(HL):
                nc.vector.memset(signv[h * 64:h * 64 + 32, :], -1.0)

            # persistent activations
            qT = [const.tile([128, N], BF16, name=f"qT_b{b}") for b in range(B)]
            kT = [const.tile([128, N], BF16, name=f"kT_b{b}") for b in range(B)]
            v1 = [const.tile([128, NKC * VW], BF16, name=f"v1_b{b}") for b in range(B)]
            for b in range(B):
                nc.vector.memset(v1[b][:, :], 1.0)  # pre-seed the ones columns

            # ---- phase 1: QKV + norm + rope ------------------------------
            with contextlib.ExitStack() as p1:
                xs = p1.enter_context(tc.tile_pool(name="xs", bufs=2))
                p1s = p1.enter_context(tc.tile_pool(name="p1s", bufs=2))
                qkps_p = p1.enter_context(tc.tile_pool(name="qkps", bufs=2, space="PSUM"))
                vps_p = p1.enter_context(tc.tile_pool(name="vps", bufs=1, space="PSUM"))
                tiny_p = p1.enter_context(tc.tile_pool(name="tiny", bufs=3, space="PSUM"))

                for t in range(B * TPB):
                    b, n0 = t // TPB, (t % TPB) * QT
                    g0 = t * QT

                    xt_f = xs.tile([128, CC * QT], F32, tag="xt_f")
                    nc.sync.dma_start(
                        out=xt_f.rearrange("p (cc n) -> p cc n", cc=CC),
                        in_=xT.rearrange("(cc p) n -> p cc n", p=128)[:, :, g0:g0 + QT])
                    xt = xs.tile([128, CC * QT], BF16, tag="xt")
                    nc.vector.tensor_copy(xt[:, :], xt_f[:, :])
                    xt_v = xt.rearrange("p (cc n) -> p cc n", cc=CC)

                    qk_ps = qkps_p.tile([128, 2 * QT], F32, tag="qk_ps")
                    for cc in range(CC):
                        nc.tensor.matmul(qk_ps[:, 0:QT], wqk_v[:, cc, 0:128],
                                         xt_v[:, cc, :], start=(cc == 0),
                                         stop=(cc == CC - 1))
                    for cc in range(CC):
                        nc.tensor.matmul(qk_ps[:, QT:2 * QT], wqk_v[:, cc, 128:256],
                                         xt_v[:, cc, :], start=(cc == 0),
                                         stop=(cc == CC - 1))

                    v_ps = vps_p.tile([128, QT], F32, tag="v_ps")
                    nsub = QT // 128
                    for sub in range(nsub):
                        for cc in range(CC):
                            nc.tensor.matmul(
                                v_ps[:, sub * 128:(sub + 1) * 128],
                                xt_v[:, cc, sub * 128:(sub + 1) * 128],
                                wv_v[:, cc, :], start=(cc == 0),
                                stop=(cc == CC - 1))
                    for sub in range(nsub):
                        cg = n0 // 128 + sub
                        dst = v1[b][:, cg * VW:(cg + 1) * VW]
                        dst = dst.rearrange("p (hl w) -> p hl w", w=65)[:, :, 0:64]
                        src = v_ps[:, sub * 128:(sub + 1) * 128]
                        src = src.rearrange("p (hl d) -> p hl d", d=64)
                        nc.vector.tensor_copy(dst, src)

                    # w-scaled raw q/k (fused with psum eviction)
                    qkw = p1s.tile([128, 2 * QT], F32, tag="qkw")
                    nc.vector.tensor_scalar_mul(qkw[:, 0:QT], qk_ps[:, 0:QT], wq_sb[:, :])
                    nc.vector.tensor_scalar_mul(qkw[:, QT:2 * QT], qk_ps[:, QT:2 * QT],
                                                wk_sb[:, :])

                    # rms statistics (1/w^2-weighted sum of squares)
                    qsq = p1s.tile([128, 2 * QT], BF16, tag="qsq")
                    nc.vector.tensor_mul(qsq[:, :], qkw[:, :], qkw[:, :])
                    ss_q = tiny_p.tile([HL, QT], F32, tag="tiny", name="ss_q")
                    ss_k = tiny_p.tile([HL, QT], F32, tag="tiny", name="ss_k")
                    nc.tensor.matmul(ss_q[:, :], blkq_bf[:, :], qsq[:, 0:QT],
                                     start=True, stop=True)
                    nc.tensor.matmul(ss_k[:, :], blkk_bf[:, :], qsq[:, QT:2 * QT],
                                     start=True, stop=True)
                    rms = p1s.tile([HL, 2 * QT], F32, tag="rms")
                    nc.scalar.activation(rms[:, 0:QT], ss_q[:, :], AF.Sqrt,
                                         scale=1.0 / D, bias=epsb[0:HL, :])
                    nc.scalar.activation(rms[:, QT:2 * QT], ss_k[:, :], AF.Sqrt,
                                         scale=1.0 / D, bias=epsb[0:HL, :])
                    rstd = p1s.tile([HL, 2 * QT], F32, tag="rstd")
                    nc.vector.reciprocal(rstd[:, :], rms[:, :])
                    rstd_bf = p1s.tile([HL, 2 * QT], BF16, tag="rstd_bf")
                    nc.vector.tensor_copy(rstd_bf[:, :], rstd[:, :])
                    bc_q = tiny_p.tile([128, QT], F32, tag="tiny", name="bc_q")
                    bc_k = tiny_p.tile([128, QT], F32, tag="tiny", name="bc_k")
                    nc.tensor.matmul(bc_q[:, :], sel_bf[:, :], rstd_bf[:, 0:QT],
                                     start=True, stop=True)
                    nc.tensor.matmul(bc_k[:, :], sel_bf[:, :], rstd_bf[:, QT:2 * QT],
                                     start=True, stop=True)

                    # rope (on w-scaled, un-normalized q/k; rstd commutes)
                    # u = x*cos, w = x*sin over all 128 partitions, then the
                    # per-head 32-row combines: or = u_r - w_i, oi = w_r + u_i
                    # rope: ror = x*cos + sign * swap(x*sin), where swap
                    # exchanges r/i 32-blocks (sin4 rows repeat per block so
                    # both inputs of each product share a base partition).
                    ror = p1s.tile([128, 2 * QT], F32, tag="ror")
                    tb = p1s.tile([128, QT], F32, tag="tb")
                    cs = cos4[:, n0:n0 + QT]
                    sn = sin4[:, n0:n0 + QT]
                    for half, c0 in ((0, 0), (1, QT)):  # q cols then k cols
                        xx = qkw[:, c0:c0 + QT]
                        ru = ror[:, c0:c0 + QT]
                        nc.vector.tensor_mul(ru, xx, cs)       # u = x*cos
                        for h in range(HL):
                            r0, i0 = h * 64, h * 64 + 32
                            nc.vector.tensor_mul(
                                tb[r0:r0 + 32, :],
                                qkw[i0:i0 + 32, c0:c0 + QT], sn[i0:i0 + 32, :])
                            nc.vector.tensor_mul(
                                tb[i0:i0 + 32, :],
                                qkw[r0:r0 + 32, c0:c0 + QT], sn[r0:r0 + 32, :])
                        nc.vector.scalar_tensor_tensor(
                            ru, tb[:, :], signv[:, :], ru,
                            op0=mybir.AluOpType.mult, op1=mybir.AluOpType.add)

                    # apply rstd broadcast + cast to bf16
                    nc.vector.tensor_mul(qT[b][:, n0:n0 + QT], ror[:, 0:QT], bc_q[:, :])
                    nc.vector.tensor_mul(kT[b][:, n0:n0 + QT], ror[:, QT:2 * QT],
                                         bc_k[:, :])

            # ---- phase 2: SDPA ------------------------------------------
            with contextlib.ExitStack() as p2:
                pts = p2.enter_context(tc.tile_pool(name="pts", bufs=3))
                p2s = p2.enter_context(tc.tile_pool(name="p2s", bufs=2))
                sc_p = p2.enter_context(tc.tile_pool(name="sc", bufs=2, space="PSUM"))
                av_p = p2.enter_context(tc.tile_pool(name="av", bufs=2, space="PSUM"))

                for b in range(B):
                    for h in range(HL):
                        for qt in range(QPB):
                            nq0 = qt * NQ
                            j = b * RPB + qt
                            av_ps = av_p.tile([65, NQ], F32, tag="av_ps")
                            for gs in range(0, NKC, GRP):
                                gl = min(GRP, NKC - gs)
                                sc_ps = sc_p.tile([128, GRP * NQ], F32, tag="sc")
                                for ci in range(gl):
                                    c = gs + ci
                                    so = sc_ps[:, ci * NQ:(ci + 1) * NQ]
                                    nc.tensor.matmul(
                                        so,
                                        kT[b][h * 64:(h + 1) * 64, c * 128:(c + 1) * 128],
                                        qT[b][h * 64:(h + 1) * 64, nq0:nq0 + NQ],
                                        start=True, stop=True)
                                pt = pts.tile([128, GRP * NQ], BF16, tag="pt")
                                nc.scalar.activation(pt[:, 0:gl * NQ],
                                                     sc_ps[:, 0:gl * NQ],
                                                     AF.Exp, bias=zero1[:, :],
                                                     scale=float(D ** -0.5))
                                for ci in range(gl):
                                    c = gs + ci
                                    nc.tensor.matmul(
                                        av_ps[:, :],
                                        v1[b][:, c * VW + h * 65:c * VW + (h + 1) * 65],
                                        pt[:, ci * NQ:(ci + 1) * NQ],
                                        start=(c == 0), stop=(c == NKC - 1))

                            rec = p2s.tile([1, NQ], F32, tag="rec")
                            nc.vector.reciprocal(rec[:, :], av_ps[64:65, :])
                            rec_bf = p2s.tile([1, NQ], BF16, tag="rec_bf")
                            nc.vector.tensor_copy(rec_bf[:, :], rec[:, :])
                            bc_o = sc_p.tile([64, NQ], F32, tag="sc", name="bc_o")
                            nc.tensor.matmul(bc_o[:, :], ones1[0:1, 0:64],
                                             rec_bf[:, :], start=True, stop=True)
                            bc_sb = p2s.tile([64, NQ], F32, tag="bc_sb")
                            nc.vector.tensor_copy(bc_sb[:, :], bc_o[:, :])
                            outT = p2s.tile([64, NQ], BF16, tag="outT")
                            nc.vector.tensor_mul(outT[:, :], av_ps[0:64, :], bc_sb[:, :])
                            nc.sync.dma_start(
                                out=a2a_in[j * 128 + h * 64:j * 128 + (h + 1) * 64, :],
                                in_=outT[:, :])

            # ---- phase 3: AllToAll --------------------------------------
            nc.gpsimd.collective_compute(
                "AllToAll", mybir.AluOpType.bypass,
                replica_groups=[list(range(R))],
                ins=[a2a_in.ap().opt()], outs=[a2a_out.ap().opt()])

            # ---- phase 4: projection ------------------------------------
            with contextlib.ExitStack() as p4:
                p4s = p4.enter_context(tc.tile_pool(name="p4s", bufs=2))
                yps_p = p4.enter_context(tc.tile_pool(name="yps", bufs=2, space="PSUM"))

                aa = const.tile([128, R * RT], BF16, name="aa")
                nc.sync.dma_start(
                    out=aa.rearrange("p (j n) -> p j n", j=R),
                    in_=a2a_out.ap().rearrange("(j p) n -> p j n", p=128))
                aa_v = aa.rearrange("p (j n) -> p j n", j=R)

                for nch in range(RT // 128):
                    for ot in range(C // OT):
                        y_ps = yps_p.tile([128, OT], F32, tag="y_ps")
                        nc.tensor.matmul(y_ps[:, :], ones1[0:1, :],
                                         bias_bf[0:1, ot * OT:(ot + 1) * OT],
                                         start=True, stop=False)
                        for j in range(R):
                            nc.tensor.matmul(
                                y_ps[:, :],
                                aa_v[:, j, nch * 128:(nch + 1) * 128],
                                wp_v[:, j, ot * OT:(ot + 1) * OT],
                                start=False, stop=(j == R - 1))
                        y_sb = p4s.tile([128, OT], F32, tag="y_sb")
                        nc.vector.tensor_copy(y_sb[:, :], y_ps[:, :])
                        nc.sync.dma_start(
                            out=y[nch * 128:(nch + 1) * 128, ot * OT:(ot + 1) * OT],
                            in_=y_sb[:, :])

    return nc


# --------------------------------------------------------------------------
# host-side sharding
# --------------------------------------------------------------------------

def shard_inputs(x, pos, qkv_w, q_norm_w, k_norm_w, proj_w, proj_b, R):
    B, N, C = x.shape
    H = C // D
    HL = H // R
    f32 = np.float32

    xT = np.ascontiguousarray(x.reshape(B * N, C).T).astype(f32, copy=False)
    Wq, Wk, Wv = qkv_w[0:C], qkv_w[C:2 * C], qkv_w[2 * C:3 * C]
    wpT = np.ascontiguousarray(proj_w.T).astype(f32, copy=False)
    biasT = np.ascontiguousarray(proj_b.reshape(1, C)).astype(f32, copy=False)
    posT = np.ascontiguousarray(pos.T).astype(f32, copy=False)      # [DH, N]
    pos4T = np.ascontiguousarray(np.concatenate([posT] * (128 // (D // 2)),
                                                axis=0))

    ev = np.arange(0, D, 2)
    od = np.arange(1, D, 2)
    perm = np.concatenate([ev, od])  # within-head feature order [evens, odds]
    wpq = np.concatenate([q_norm_w[perm]] * HL)
    wpk = np.concatenate([k_norm_w[perm]] * HL)
    wpq = np.ascontiguousarray(wpq.reshape(HL * D, 1)).astype(f32, copy=False)
    wpk = np.ascontiguousarray(wpk.reshape(HL * D, 1)).astype(f32, copy=False)

    sel = np.zeros((HL, HL * D), np.float32)
    for h in range(HL):
        sel[h, h * D:(h + 1) * D] = 1.0

    in_maps = []
    for r in range(R):
        heads = [r * HL + i for i in range(HL)]

        def qk_rows(W):
            rows = [W[h * D + perm] for h in heads]
            return np.concatenate(rows, axis=0)  # [HL*D, C]

        wqkT = np.ascontiguousarray(
            np.concatenate([qk_rows(Wq), qk_rows(Wk)], axis=0).T).astype(f32)
        wvT = np.ascontiguousarray(
            np.concatenate([Wv[h * D:(h + 1) * D] for h in heads], axis=0).T
        ).astype(f32)
        in_maps.append(dict(xT=xT, wqkT=wqkT, wvT=wvT, wpT=wpT, biasT=biasT,
                            pos4T=pos4T, wpq=wpq, wpk=wpk, selin=sel))
    return in_maps


# --------------------------------------------------------------------------
# entry point
# --------------------------------------------------------------------------

_CACHE = {}


def _get_program(cfg_key, **cfg):
    if cfg_key not in _CACHE:
        nc = build_program(**cfg)
        nc.compile()
        _CACHE[cfg_key] = nc
    return _CACHE[cfg_key]


def kernel(x, pos, qkv_w, q_norm_w, k_norm_w, proj_w, proj_b, _trace=False):
    cfg = CFG_FULL
    B, N, C, R = cfg["B"], cfg["N"], cfg["C"], cfg["R"]
    assert x.shape == (B, N, C)

    in_maps = shard_inputs(np.asarray(x, np.float32), np.asarray(pos, np.float32),
                           np.asarray(qkv_w, np.float32),
                           np.asarray(q_norm_w, np.float32),
                           np.asarray(k_norm_w, np.float32),
                           np.asarray(proj_w, np.float32),
                           np.asarray(proj_b, np.float32), R)
    nc = _get_program("full", **cfg)
    res = bass_utils.run_bass_kernel_spmd(nc, in_maps, core_ids=list(range(R)),
                                          trace=_trace)
    out = np.concatenate([res.results[r]["y"] for r in range(R)], axis=0)
    out = out.reshape(B, N, C).astype(np.float32, copy=False)
    if _trace:
        kernel.last_exec_time_ns = res.exec_time_ns
        kernel.last_results = res
    return out
